# revision 57
# baseline (speedup 1.0000x reference)
"""Trainium2 Bass kernel for the NLNN (non-local neural network) block.

Reference semantics (per batch b, with X = x[b] as [1024, 2304] and N = 48*48):
    T   = w1 @ X            [512, 2304]
    PHI = w2 @ X            [512, 2304]
    G   = w3 @ X            [512, 2304]
    T'  = reshape(T,  [2304, 512])   (raw row-major memory reinterpretation)
    G'  = reshape(G,  [2304, 512])
    A   = softmax(T' @ PHI, axis=-1) [2304, 2304]
    Y   = A @ G'            [2304, 512]
    Yr  = reshape(Y, [512, 2304])
    out = X + w4 @ Yr + b4  [1024, 2304]

Sharding: pure data parallelism — batch B=8 mapped 1:1 onto 8 NeuronCores.

On-chip strategy (per core):
  - theta/phi convs and the logits matmul run in bf16 (precision-critical:
    exp amplifies logit error on this very peaked softmax; fp8 logits
    measured 1.6e-1 rel err vs 1.1e-2 for this scheme).
  - g, exp(att), y and w4 are quantized to fp8-e4m3 so the Y matmul and
    the final conv run in DoubleRow mode (2 fp8 rows/PE cell, 2x).
  - exp(att^T) is cast to fp8 with per-column scale 128/den, den being
    the pre-quantization softmax denominator: never clips (top weight
    <= 1 maps to <= 128), and the columns of ae8 then sum to ~128, so y
    normalization is the constant 2^-7 on the Scalar engine. den comes
    from ones-lhsT matmuls interleaved with the logits (full 128-wide
    ones replicates den into every partition, so no cross-partition
    reduce is needed); DVE only does one reciprocal + 18 fused
    scale-casts per strip.
  - Strip emission is software-pipelined (logits s+1 before Y s) so the
    Tensor engine never waits on the cast machinery.
  - The final conv preloads the residual x*2^13 into PSUM on DVE (off
    the yr8 critical path), accumulates the fp8 DoubleRow matmuls on
    top, and finishes with a single Scalar scale-out per chunk.
  - The awkward 4.5-ratio reshapes (T->T', G->G', Y->Yr) are realized by
    HBM round trips with natural access patterns; T' additionally gets the
    DMA xbar transpose. y/Yr round-trips through four per-row-block fp8
    tiles so each Yr read only waits on the strips that feed it.
  - b4 is folded into the residual x host-side, so the epilogue is a
    single DVE (psum * 2^-13 + x_in) op; the 2^13 is the fp8 scale
    product of y (x16) and w4 (x512).
"""

import numpy as np
import ml_dtypes

import concourse.bass as bass
import concourse.bacc as bacc
import concourse.mybir as mybir
import concourse.tile as tile
from concourse.bass_utils import run_bass_kernel_spmd

F32 = mybir.dt.float32
BF16 = mybir.dt.bfloat16
F8 = mybir.dt.float8e4
AF = mybir.ActivationFunctionType
ALU = mybir.AluOpType
PM = mybir.MatmulPerfMode

C_IN = 1024
C_MID = 512
H = W = 48
N = H * W  # 2304
B = 8
NCORES = 8
KT = C_IN // 128   # 8  k tiles over input channels
MT = C_MID // 128  # 4  tiles over mid channels
NT = N // 128      # 18 tiles over spatial dim
# free-dim chunks of <=512 (one fp32 PSUM bank)
NCHUNKS = [(i, min(i + 512, N)) for i in range(0, N, 512)]
NROWS = N // MT    # 576 y rows per Yr row-tile
GW = 528           # padded gaug row width (16-aligned for DoubleRow steps)


def _emit(nc, tc, t_in, t_out):
    x_d = t_in["x"]

    with (
        tc.tile_pool(name="mega", bufs=1) as mega,
        tc.tile_pool(name="psum", bufs=6, space="PSUM") as psp,
        tc.tile_pool(name="dram", bufs=1, space="DRAM") as dramp,
        tc.tile_pool(name="small", bufs=4) as smallp,
    ):
        # ---- long-lived tiles (slots are re-tagged across phases) ----
        phi = mega.tile([128, MT, N], BF16, tag="phi")
        ttT = mega.tile([128, MT, N], BF16, tag="ttT")       # T'^T, [c, n]
        gaug = mega.tile([128, NT, 512], F8, tag="gaug")     # G'*16, fp8
        w4s8 = mega.tile([128, MT, C_IN], F8, tag="w4s")     # 512*w4, fp8
        bsml = mega.tile([128, 3 * MT], F32, tag="bsml")     # b1|b2|16*b3

        # flat HBM intermediates implementing the raw reshapes
        t_dram = dramp.tile([C_MID * N], BF16, tag="t_dram")
        g_dram = dramp.tile([C_MID * N], F8, tag="g_dram")
        y_dram = [dramp.tile([NROWS * C_MID], F8, tag=f"y_dram{rt}",
                             name=f"y_dram{rt}")
                  for rt in range(MT)]
        t_w = t_dram[:].rearrange("(t p m) -> p t m", p=128, m=N)
        t_r = t_dram[:].rearrange("(n c) -> n c", c=C_MID)  # T' view [2304, 512]
        g_w = g_dram[:].rearrange("(t p m) -> p t m", p=128, m=N)
        g_r = g_dram[:].rearrange("(t p c) -> p t c", p=128, c=C_MID)  # G' tiles
        y_w = [yd[:].rearrange("(n c) -> n c", c=C_MID) for yd in y_dram]
        y_r = [yd[:].rearrange("(p m) -> p m", p=128) for yd in y_dram]

        # head loads: w1 k-slices interleaved with xb chunk-0 k-slices so the
        # first matmul can start after ~256KB instead of ~3.5MB.
        w1s = mega.tile([128, KT, C_MID], BF16, tag="w1s")
        xb = mega.tile([128, KT * N], BF16, tag="xmem")

        def xbr(ci, k):
            n0, n1 = NCHUNKS[ci]
            return xb[:, KT * n0 + k * (n1 - n0):KT * n0 + (k + 1) * (n1 - n0)]

        def load_xb(ci):
            n0, n1 = NCHUNKS[ci]
            mid = KT * n0 + (KT * (n1 - n0)) // 2
            nc.sync.dma_start(xb[:, KT * n0:mid], t_in["xb"][:, KT * n0:mid])
            nc.sync.dma_start(xb[:, mid:KT * n1], t_in["xb"][:, mid:KT * n1])

        w1v = t_in["w1t"][:].rearrange("p (t c) -> p t c", c=C_MID)
        # finest-first: the opening matmul only needs w1 k0/mb0 (32KB) and
        # the first half of xbr(0,0) (64KB) -- land those on separate queues
        # so compute starts ~6us earlier than with 256KB-granular loads.
        nc.sync.dma_start(w1s[:, 0:1, 0:128], w1v[:, 0:1, 0:128])
        nc.sync.dma_start(xb[:, 0:256], t_in["xb"][:, 0:256])
        nc.sync.dma_start(w1s[:, 0:1, 128:512], w1v[:, 0:1, 128:512])
        nc.sync.dma_start(xb[:, 256:512], t_in["xb"][:, 256:512])
        nc.sync.dma_start(w1s[:, 1:2, :], w1v[:, 1:2, :])
        nc.sync.dma_start(xb[:, 512:1024], t_in["xb"][:, 512:1024])
        for k in range(2, KT, 2):
            nc.sync.dma_start(w1s[:, k:k + 2, :], w1v[:, k:k + 2, :])
            nc.sync.dma_start(xb[:, 512 * k:512 * (k + 2)],
                              t_in["xb"][:, 512 * k:512 * (k + 2)])
        nc.sync.dma_start(bsml[:], t_in["bpack"][:])
        load_xb(1)
        load_xb(2)
        load_xb(3)
        load_xb(4)
        w2s = mega.tile([128, KT, C_MID], BF16, tag="w2s")
        nc.sync.dma_start(w2s[:], t_in["w2t"][:].rearrange("p (t c) -> p t c", c=C_MID))
        w3s = mega.tile([128, KT, C_MID], F8, tag="w3s")
        nc.sync.dma_start(w3s[:], t_in["w3t"][:].rearrange("p (t c) -> p t c", c=C_MID))

        # fp8 copy of x (x16) for the g conv, cast on DVE while the theta/phi
        # convs own the Tensor engine. Pair-contiguous [ci, k2, j, c] layout
        # so DoubleRow rhs slices are contiguous; 256-wide tail chunk apart.
        xb8m = mega.tile([128, 4, KT // 2, 2, 512], F8, tag="xb8m")
        xb8e = mega.tile([128, KT // 2, 2, 256], F8, tag="xb8e")
        for ci in range(len(NCHUNKS)):
            for k in range(KT):
                if ci < 4:
                    nc.vector.tensor_scalar_mul(
                        xb8m[:, ci, k // 2, k % 2, :], xbr(ci, k), 16.0)
                else:
                    nc.vector.tensor_scalar_mul(
                        xb8e[:, k // 2, k % 2, :], xbr(ci, k), 16.0)

        def conv(ws, boff, dest_sb, scale):
            """dest = scale*(w.T @ xb + bias); k-outer so chunk 0 can start
            on its first k-slices as they arrive."""
            for ci, (n0, n1) in enumerate(NCHUNKS):
                pss = []
                for mb in range(MT):
                    pss.append(psp.tile([128, n1 - n0], F32, tag="ps", name="ps"))
                for k in range(KT):
                    for mb in range(MT):
                        nc.tensor.matmul(
                            pss[mb][:],
                            lhsT=ws[:, k, mb * 128:(mb + 1) * 128],
                            rhs=xbr(ci, k),
                            start=(k == 0),
                            stop=(k == KT - 1),
                        )
                for mb in range(MT):
                    nc.scalar.activation(dest_sb[:, mb, n0:n1], pss[mb][:],
                                         AF.Identity, scale=scale,
                                         bias=bsml[:, boff * MT + mb:boff * MT + mb + 1])

        def conv8(ws8, boff, dest_sb, scale):
            """fp8 DoubleRow conv: dest = scale*(w8.T @ xb8) + bias."""
            for ci, (n0, n1) in enumerate(NCHUNKS):
                pss = []
                for mb in range(MT):
                    pss.append(psp.tile([128, n1 - n0], F32, tag="ps", name="ps"))
                for k2 in range(KT // 2):
                    rhs = (xb8m[:, ci, k2, :, :] if ci < 4
                           else xb8e[:, k2, :, :])
                    for mb in range(MT):
                        nc.tensor.matmul(
                            pss[mb][:],
                            lhsT=ws8[:, 2 * k2:2 * k2 + 2, mb * 128:(mb + 1) * 128],
                            rhs=rhs,
                            start=(k2 == 0),
                            stop=(k2 == KT // 2 - 1),
                            perf_mode=PM.DoubleRow,
                        )
                for mb in range(MT):
                    nc.scalar.activation(dest_sb[:, mb, n0:n1], pss[mb][:],
                                         AF.Identity, scale=scale,
                                         bias=bsml[:, boff * MT + mb:boff * MT + mb + 1])

        # theta conv first: its HBM round trip overlaps phi/g convs
        tstg = mega.tile([128, MT, N], BF16, tag="ae", bufs=2, name="tstg")
        conv(w1s, 0, tstg, 1.0)
        nc.sync.dma_start(t_w, tstg[:])
        # T'^T via xbar transpose reads of the flat T buffer
        for ct in range(MT):
            nc.sync.dma_start(
                ttT[:, ct, :], t_r[:, ct * 128:(ct + 1) * 128], transpose=True
            )
        conv(w2s, 1, phi, 1.0)
        # g conv emits 16*g (bias pre-scaled host-side) directly in fp8 so
        # the Y matmul can run in DoubleRow mode; the conv itself runs fp8
        # DoubleRow (w3 x2048, x x16 -> psum 32768*g, scale 2^-11 = 16/32768)
        gstg = mega.tile([128, MT, N], F8, tag="ae", bufs=2, name="gstg")
        conv8(w3s, 2, gstg, 2.0 ** -11)
        nc.sync.dma_start(g_w[:, 0:2, :], gstg[:, 0:2, :])
        nc.sync.dma_start(gaug[:, 0:9, 0:512], g_r[:, 0:9, :])
        nc.sync.dma_start(g_w[:, 2:4, :], gstg[:, 2:4, :])
        nc.sync.dma_start(gaug[:, 9:18, 0:512], g_r[:, 9:18, :])

        # phase-E constants
        nc.sync.dma_start(w4s8[:], t_in["w4t"][:].rearrange("p (t c) -> p t c", c=C_IN))

        # fp32 x (with b4 pre-added host-side) for the residual
        NXF = 8
        xf = mega.tile([128, NXF, N], BF16, tag="xmem")

        # Yr row-tile PAIRS (DoubleRow moving operands), in dead conv-w slots.
        # Layout is per-chunk pair-contiguous [ci, j, c] so every DoubleRow
        # rhs slice [2, w] is contiguous (strided pairs halve the moving
        # rate); the 256-wide tail chunk lives in its own tile.
        yr8m = [mega.tile([128, 4, 2, 512], F8, tag=("w1s", "w2s")[i],
                          name="yr8m_t") for i in range(2)]
        yr8e = mega.tile([128, 2, 2, 256], F8, tag="yr8e")

        def load_yr8(i, j, p0=0, p1=128):
            src = y_r[2 * i + j]
            nc.sync.dma_start(yr8m[i][p0:p1, :, j, :], src[p0:p1, 0:2048])
            nc.sync.dma_start(yr8e[p0:p1, i, j, :], src[p0:p1, 2048:2304])

        def write_y8(ng, y_t, halves=False):
            r0 = ng * 128
            rt = r0 // NROWS
            split = (rt + 1) * NROWS - r0  # rows in this tile (<=128)
            if split >= 128:
                rr = r0 - rt * NROWS
                if halves:
                    # tail-critical: land both halves on parallel queues
                    nc.sync.dma_start(y_w[rt][rr:rr + 64, :], y_t[0:64, :])
                    nc.sync.dma_start(y_w[rt][rr + 64:rr + 128, :], y_t[64:128, :])
                else:
                    nc.sync.dma_start(y_w[rt][rr:rr + 128, :], y_t[:])
            else:
                nc.sync.dma_start(y_w[rt][r0 - rt * NROWS:, :], y_t[0:split, :])
                nc.sync.dma_start(y_w[rt + 1][0:128 - split, :], y_t[split:128, :])

        # ---- attention + Y, strip by strip over n ----
        # Software-pipelined emission: Tensor stream is
        #   logits(0), logits(1), Y(0), logits(2), Y(1), ... Y(4)
        # so the Tensor engine never waits on the fp8 cast of the strip it is
        # about to consume; the cast machinery for strip s runs on DVE
        # underneath logits(s+1).
        #
        # fp8 scale is 240/den (den = sum_m ae[m, n], the pre-quantization
        # softmax denominator): never clips (top weight <= 1), underflow
        # truncates only weights < 2^-9/240 of the column sum. den comes from
        # ones-lhsT matmuls interleaved with the logits (full 128-wide ones
        # so every partition holds den[n] -- no cross-partition reduce).
        ones128 = mega.tile([128, 128], BF16, tag="ones")
        nc.vector.memset(ones128[:], 1.0)
        strip_state = {}

        def emit_logits_den(si):
            n0, n1 = NCHUNKS[si]
            wn = n1 - n0
            ae = mega.tile([128, NT, wn], BF16, tag="ae", bufs=2, name="ae")
            ae8 = mega.tile([128, NT, wn], F8, tag="ae8", bufs=2, name="ae8")
            rct = smallp.tile([128, wn], F32, tag="rct", bufs=2, name="rct")
            psD = psp.tile([128, wn], F32, tag="psd", bufs=2, name="psD")
            strip_state[si] = (ae, ae8, rct, psD)

            def den(mb):
                nc.tensor.matmul(psD[:], lhsT=ones128[:], rhs=ae[:, mb, :],
                                 start=(mb == 0), stop=(mb == NT - 1))

            for ci in range(si * 2, min(si * 2 + 2, NXF)) if si < 4 else range(0):
                nc.sync.dma_start(xf[:, ci, :], x_d[ci * 128:(ci + 1) * 128, :])
            for mb in range(NT):
                ps = psp.tile([128, wn], F32, tag="ps")
                for ct in range(MT):
                    nc.tensor.matmul(
                        ps[:],
                        lhsT=phi[:, ct, mb * 128:(mb + 1) * 128],
                        rhs=ttT[:, ct, n0:n1],
                        start=(ct == 0),
                        stop=(ct == MT - 1),
                    )
                nc.scalar.activation(ae[:, mb, :], ps[:], AF.Exp)
                # den(mb) one logits-group later so exp(mb) is done when the
                # Tensor engine reaches it
                if mb > 0:
                    den(mb - 1)
            den(NT - 1)

        def emit_casts(si):
            # ae8 = ae * 240/den; the per-column scale cancels exactly in
            # the softmax ratio since the ones-column denominator is scaled
            # identically.
            ae, ae8, rct, psD = strip_state[si]
            nc.vector.reciprocal(rct[:], psD[:])
            for mb in range(NT):
                nc.vector.scalar_tensor_tensor(
                    ae8[:, mb, :], ae[:, mb, :], 128.0, rct[:],
                    op0=ALU.mult, op1=ALU.mult)

        def emit_y(si):
            n0, n1 = NCHUNKS[si]
            wn = n1 - n0
            ae8 = strip_state[si][1]
            for nbl in range(wn // 128):
                psY = psp.tile([128, C_MID], F32, tag="ps")
                for t in range(NT // 2):
                    nc.tensor.matmul(
                        psY[:],
                        lhsT=ae8[:, 2 * t:2 * t + 2, nbl * 128:(nbl + 1) * 128],
                        rhs=gaug[:, 2 * t:2 * t + 2, :],
                        start=(t == 0), stop=(t == NT // 2 - 1),
                        perf_mode=PM.DoubleRow)
                # ae8 columns sum to ~128 by construction (scale 128/den), so
                # y normalization is the constant 2^-7; g carries x16, making
                # y_t directly 16*y in fp8.
                y_t = smallp.tile([128, C_MID], F8, tag="yt")
                nc.scalar.activation(y_t[:], psY[:], AF.Identity,
                                     scale=2.0 ** -7)
                write_y8(n0 // 128 + nbl, y_t, halves=(si == 4))
                if si == 4 and nbl == 0:
                    # partitions 64-98 cover y rows <= 2173 (written by
                    # strips 3 + 4/nbl0); partition 99 crosses into nbl1
                    load_yr8(1, 1, 64, 82)
                    load_yr8(1, 1, 82, 99)
                elif si == 4 and nbl == 1:
                    load_yr8(1, 1, 99, 114)
                    load_yr8(1, 1, 114, 128)
            # stagger Yr pair-tile quarter loads right after the strip that
            # completes their source rows
            if si == 1:
                load_yr8(0, 0)
            elif si == 2:
                load_yr8(0, 1)
            elif si == 3:
                load_yr8(1, 0)
                # first half of rt3 (y rows 1728-2015) is complete after
                # strip 3 as well
                load_yr8(1, 1, 0, 64)
            # (si == 4 tail reads are emitted inside the nbl loop above)

        nstrips = len(NCHUNKS)
        emit_logits_den(0)
        emit_casts(0)
        for si in range(nstrips):
            if si + 1 < nstrips:
                emit_logits_den(si + 1)
            emit_y(si)
            if si + 1 < nstrips:
                emit_casts(si + 1)

        # ---- final conv + residual: out = x_in + w4 @ Yr  (b4 in x_in) ----
        for cb in range(KT):
            xcb = xf[:, cb, :]
            out_t = mega.tile([128, N], BF16,
                              tag=("ttT", "phi", "gaug", "w3s", "xb8m")[cb % 5],
                              name="out_t")
            for ci, (n0, n1) in enumerate(NCHUNKS):
                # balance the epilogue across DVE (one fused op) and Scalar
                # (residual preloaded into PSUM + scale-out) ~7:3 so neither
                # engine paces the final conv.
                on_dve = (cb * len(NCHUNKS) + ci) % 10 < 7
                ps = psp.tile([128, n1 - n0], F32, tag="ps", name="ps")
                if not on_dve:
                    nc.scalar.activation(ps[:], xcb[:, n0:n1], AF.Identity,
                                         scale=2.0 ** 13)
                rhs0 = yr8m[0][:, ci, :, :] if ci < 4 else yr8e[:, 0, :, :]
                rhs1 = yr8m[1][:, ci, :, :] if ci < 4 else yr8e[:, 1, :, :]
                nc.tensor.matmul(ps[:], lhsT=w4s8[:, 0:2, cb * 128:(cb + 1) * 128],
                                 rhs=rhs0,
                                 start=on_dve, stop=False, perf_mode=PM.DoubleRow,
                                 skip_group_check=True)
                nc.tensor.matmul(ps[:], lhsT=w4s8[:, 2:4, cb * 128:(cb + 1) * 128],
                                 rhs=rhs1,
                                 start=False, stop=True, perf_mode=PM.DoubleRow,
                                 skip_group_check=True)
                if on_dve:
                    nc.vector.scalar_tensor_tensor(
                        out_t[:, n0:n1], ps[:], 2.0 ** -13,
                        xcb[:, n0:n1], op0=ALU.mult, op1=ALU.add)
                else:
                    nc.scalar.activation(out_t[:, n0:n1], ps[:], AF.Identity,
                                         scale=2.0 ** -13)
                if cb < KT - 1:
                    if n1 == 1024:
                        nc.sync.dma_start(t_out[cb * 128:(cb + 1) * 128, 0:1024],
                                          out_t[:, 0:1024])
                else:
                    nc.sync.dma_start(t_out[cb * 128:(cb + 1) * 128, n0:n1],
                                      out_t[:, n0:n1])
            if cb < KT - 1:
                nc.sync.dma_start(t_out[cb * 128:(cb + 1) * 128, 1024:N],
                                  out_t[:, 1024:N])


def build_module():
    nc = bacc.Bacc("TRN2", target_bir_lowering=False, debug=False)
    t_in = {
        "x": nc.dram_tensor("x", [C_IN, N], BF16, kind="ExternalInput").ap(),
        "xb": nc.dram_tensor("xb", [128, KT * N], BF16, kind="ExternalInput").ap(),
        "w1t": nc.dram_tensor("w1t", [128, KT * C_MID], BF16, kind="ExternalInput").ap(),
        "w2t": nc.dram_tensor("w2t", [128, KT * C_MID], BF16, kind="ExternalInput").ap(),
        "w3t": nc.dram_tensor("w3t", [128, KT * C_MID], F8, kind="ExternalInput").ap(),
        "w4t": nc.dram_tensor("w4t", [128, MT * C_IN], F8, kind="ExternalInput").ap(),
        "bpack": nc.dram_tensor("bpack", [128, 3 * MT], F32, kind="ExternalInput").ap(),
    }
    t_out = nc.dram_tensor("out", [C_IN, N], BF16, kind="ExternalOutput").ap()
    with tile.TileContext(nc) as tc:
        _emit(nc, tc, t_in, t_out)
    nc.compile()
    return nc


_NC = None


def _get_nc():
    global _NC
    if _NC is None:
        _NC = build_module()
    return _NC


def _ptile(a):
    """[T*128, C] -> [128, T*C] with the 128-partition dim outermost."""
    t = a.shape[0] // 128
    return np.ascontiguousarray(
        a.reshape(t, 128, a.shape[1]).transpose(1, 0, 2).reshape(128, -1)
    )


def make_in_maps(x, w1, b1, w2, b2, w3, b3, w4, b4):
    bf = ml_dtypes.bfloat16
    f8 = ml_dtypes.float8_e4m3
    bpack = np.stack(
        [np.asarray(b1, np.float32).reshape(MT, 128),
         np.asarray(b2, np.float32).reshape(MT, 128),
         16.0 * np.asarray(b3, np.float32).reshape(MT, 128)],
        axis=0,
    ).transpose(2, 0, 1).reshape(128, 3 * MT)
    shared = {
        "w1t": _ptile(np.asarray(w1, np.float32).T).astype(bf),
        "w2t": _ptile(np.asarray(w2, np.float32).T).astype(bf),
        "w3t": np.clip(_ptile(np.asarray(w3, np.float32).T) * 2048.0,
                       -240.0, 240.0).astype(f8),
        "w4t": np.clip(_ptile(np.asarray(w4, np.float32).T) * 512.0,
                       -240.0, 240.0).astype(f8),
        "bpack": np.ascontiguousarray(bpack),
    }
    x = np.asarray(x, np.float32)
    xpb = x.reshape(B, C_IN, N) + np.asarray(b4, np.float32)[None, :, None]
    maps = []
    for i in range(B):
        xi = np.ascontiguousarray(x[i].reshape(C_IN, N))
        x8 = xi.reshape(KT, 128, N)
        xbt = np.concatenate(
            [x8[:, :, n0:n1].transpose(1, 0, 2).reshape(128, -1)
             for (n0, n1) in NCHUNKS], axis=1)
        maps.append({"x": np.ascontiguousarray(xpb[i]).astype(bf),
                     "xb": np.ascontiguousarray(xbt).astype(bf), **shared})
    return maps


def _run(in_maps, **kw):
    return run_bass_kernel_spmd(_get_nc(), in_maps, list(range(NCORES)), **kw)


def kernel(x, w1, b1, w2, b2, w3, b3, w4, b4):
    res = _run(make_in_maps(x, w1, b1, w2, b2, w3, b3, w4, b4))
    out = np.stack([np.asarray(res.results[i]["out"]) for i in range(B)])
    return out.reshape(B, C_IN, H, W).astype(np.float32)

